# revision 1
# baseline (speedup 1.0000x reference)
"""Trainium2 Bass kernel for nn_AttentionBlock_15693810500077.

GroupNorm(32 groups) -> 1x1 qkv conv -> 4-head attention (T=4096) ->
1x1 proj -> residual, for x [2, 256, 16, 16, 16] fp32.

Sharding: 8 cores = (batch b in {0,1}) x (t-slice i in {0..3}, TS=1024).
Each core computes the full attention rows for its t-slice of its batch,
for all 4 heads, plus the projection and residual -> y^T slab [1024, 256].
The host rotates each core's x copy (np.roll over T) so the core's t-slice
always sits at columns 0:1024 -> one static SPMD program for all cores
(softmax over keys is permutation invariant).

Self-contained: hardcodes all shapes; only needs numpy + the concourse
(Bass) runtime available in the environment.
"""
import os

import numpy as np

os.environ.setdefault("JAX_COMPILATION_CACHE_DIR", "/tmp/jaxcache")

import concourse.bass as bass
import concourse.tile as tile
from concourse import mybir
from concourse.bass_utils import run_bass_kernel_spmd

F32 = mybir.dt.float32
F32R = mybir.dt.float32r
BF16 = mybir.dt.bfloat16
AF = mybir.ActivationFunctionType
ALU = mybir.AluOpType

H = 4
C = 256
T = 4096
TS = 1024
EPS = 1e-5
SCALE2 = 0.125           # (1/sqrt(sqrt(64)))^2
NCHUNKS = T // 128       # 32 key chunks of 128


def _mm(nc, out, lhsT, rhs, start=True, stop=True, r=True):
    """matmul with fp32r bitcast and N<=512 chunking along the free dim."""
    n = rhs.free_size()
    lt = lhsT.bitcast(F32R) if r else lhsT
    for n0 in range(0, n, 512):
        n1 = min(n0 + 512, n)
        rh = rhs[:, n0:n1]
        nc.tensor.matmul(
            out[:, n0:n1],
            lt,
            rh.bitcast(F32R) if r else rh,
            start=start,
            stop=stop,
        )


def build_nc():
    nc = bass.Bass()

    x_d = nc.dram_tensor("x", [C, T], F32, kind="ExternalInput")
    xT_d = nc.dram_tensor("xT", [TS, C], F32, kind="ExternalInput")
    wqT_d = nc.dram_tensor("wqT", [C, C], BF16, kind="ExternalInput")
    wkT_d = nc.dram_tensor("wkT", [C, C], BF16, kind="ExternalInput")
    wvT_d = nc.dram_tensor("wvT", [C, C], BF16, kind="ExternalInput")
    pT_d = nc.dram_tensor("pT", [4, 64, C], BF16, kind="ExternalInput")
    normw_d = nc.dram_tensor("normw", [2, 128, 1], F32, kind="ExternalInput")
    normb_d = nc.dram_tensor("normb", [2, 128, 1], F32, kind="ExternalInput")
    projb_d = nc.dram_tensor("projb", [1, C], F32R, kind="ExternalInput")
    sel_d = nc.dram_tensor("sel", [128, 16], F32, kind="ExternalInput")
    exp_d = nc.dram_tensor("expand", [16, 128], F32, kind="ExternalInput")
    ones_d = nc.dram_tensor("ones", [128, 128], F32R, kind="ExternalInput")
    onesb_d = nc.dram_tensor("onesb", [128, 128], BF16, kind="ExternalInput")
    yT_d = nc.dram_tensor("yT", [TS, C], F32, kind="ExternalOutput")

    import contextlib

    with tile.TileContext(nc) as tc:
        with (
            tc.tile_pool(name="consts", bufs=1) as consts,
            tc.tile_pool(name="gnp", bufs=2) as gnp,
            tc.tile_pool(name="kqv", bufs=1) as kqv,
            tc.tile_pool(name="psA", bufs=2, space="PSUM") as psA,
            tc.tile_pool(name="psB", bufs=2, space="PSUM") as psB,
            contextlib.ExitStack() as late,
        ):
            # ---- constant / weight loads ----
            wq = [consts.tile([128, C], BF16, name=f"wq{i}") for i in range(2)]
            wk = [consts.tile([128, C], BF16, name=f"wk{i}") for i in range(2)]
            wv = [consts.tile([128, C], BF16, name=f"wv{i}") for i in range(2)]
            for i in range(2):
                nc.sync.dma_start(out=wq[i], in_=wqT_d[i * 128:(i + 1) * 128, :])
                nc.sync.dma_start(out=wk[i], in_=wkT_d[i * 128:(i + 1) * 128, :])
                nc.sync.dma_start(out=wv[i], in_=wvT_d[i * 128:(i + 1) * 128, :])
            pT = [consts.tile([64, C], BF16, name=f"pT{h}") for h in range(H)]
            for h in range(H):
                nc.sync.dma_start(out=pT[h], in_=pT_d[h])
            normw = [consts.tile([128, 1], F32, name=f"nw{i}") for i in range(2)]
            normb = [consts.tile([128, 1], F32, name=f"nb{i}") for i in range(2)]
            for i in range(2):
                nc.sync.dma_start(out=normw[i], in_=normw_d[i])
                nc.sync.dma_start(out=normb[i], in_=normb_d[i])
            projb = consts.tile([1, C], F32R, name="projb")
            nc.sync.dma_start(out=projb, in_=projb_d[:])
            sel = consts.tile([128, 16], F32, name="sel")
            nc.sync.dma_start(out=sel, in_=sel_d[:])
            expand = consts.tile([16, 128], F32, name="expand")
            nc.sync.dma_start(out=expand, in_=exp_d[:])
            xT_sb = consts.tile([128, 8, C], F32, name="xT_sb")
            nc.sync.dma_start(
                out=xT_sb, in_=xT_d.rearrange("(a p) o -> p a o", p=128)
            )
            ones = consts.tile([128, 128], F32R, name="ones")
            nc.sync.dma_start(out=ones, in_=ones_d[:])

            # ---- load x, GroupNorm -> xn ----
            xn = [kqv.tile([128, T], BF16, name=f"xn{i}") for i in range(2)]
            with tc.tile_pool(name="xp", bufs=1) as xp:
                xt = [xp.tile([128, T], F32, name=f"x{i}") for i in range(2)]
                for i in range(2):
                    for jc in range(4):
                        nc.sync.dma_start(
                            out=xt[i][:, jc * 1024:(jc + 1) * 1024],
                            in_=x_d[i * 128:(i + 1) * 128,
                                    jc * 1024:(jc + 1) * 1024],
                        )
                for i in range(2):
                    xv = xt[i].rearrange("p (a f) -> p a f", f=512)
                    stats = gnp.tile([128, 8, 6], F32, name="stats", tag="stats")
                    for j in range(8):
                        nc.vector.bn_stats(out=stats[:, j, :], in_=xv[:, j, :])
                    mv = gnp.tile([128, 2], F32, name="mv", tag="mv")
                    nc.vector.bn_aggr(out=mv, in_=stats)
                    # exsq = var + mean^2
                    msq = gnp.tile([128, 1], F32, name="msq", tag="msq")
                    nc.vector.tensor_mul(msq, mv[:, 0:1], mv[:, 0:1])
                    exsq = gnp.tile([128, 1], F32, name="exsq", tag="exsq")
                    nc.vector.tensor_add(exsq, msq, mv[:, 1:2])
                    # group stats via selector matmuls (plain fp32, tiny)
                    gm_ps = psB.tile([16, 1], F32, name="gm_ps", tag="acc")
                    nc.tensor.matmul(gm_ps, sel, mv[:, 0:1], start=True, stop=True)
                    gx_ps = psB.tile([16, 1], F32, name="gx_ps", tag="acc")
                    nc.tensor.matmul(gx_ps, sel, exsq, start=True, stop=True)
                    gm_sb = gnp.tile([16, 1], F32, name="gm_sb", tag="gm_sb")
                    nc.vector.tensor_copy(gm_sb, gm_ps)
                    gmsq = gnp.tile([16, 1], F32, name="gmsq", tag="gmsq")
                    nc.vector.tensor_mul(gmsq, gm_sb, gm_sb)
                    gvar = gnp.tile([16, 1], F32, name="gvar", tag="gvar")
                    nc.vector.scalar_tensor_tensor(
                        gvar, gx_ps, EPS, gmsq, op0=ALU.add, op1=ALU.subtract
                    )
                    # rstd = exp(-0.5 * ln(var + eps))
                    lnv = gnp.tile([16, 1], F32, name="lnv", tag="lnv")
                    nc.scalar.activation(lnv, gvar, AF.Ln)
                    rstd = gnp.tile([16, 1], F32, name="rstd", tag="rstd")
                    nc.scalar.activation(rstd, lnv, AF.Exp, scale=-0.5)
                    # expand to channels
                    me_ps = psB.tile([128, 1], F32, name="me_ps", tag="acc")
                    nc.tensor.matmul(me_ps, expand, gm_sb, start=True, stop=True)
                    re_ps = psB.tile([128, 1], F32, name="re_ps", tag="acc")
                    nc.tensor.matmul(re_ps, expand, rstd, start=True, stop=True)
                    a_sb = gnp.tile([128, 1], F32, name="a_sb", tag="a_sb")
                    nc.vector.tensor_mul(a_sb, re_ps, normw[i])
                    t2 = gnp.tile([128, 1], F32, name="t2", tag="t2")
                    nc.vector.tensor_mul(t2, me_ps, a_sb)
                    b_sb = gnp.tile([128, 1], F32, name="b_sb", tag="b_sb")
                    nc.vector.tensor_sub(b_sb, normb[i], t2)
                    nc.vector.tensor_scalar(
                        out=xn[i], in0=xt[i], scalar1=a_sb, scalar2=b_sb,
                        op0=ALU.mult, op1=ALU.add,
                    )

            # ---- late pools (opened after the x pool is released) ----
            ppool = late.enter_context(tc.tile_pool(name="ppool", bufs=3))
            rsp = late.enter_context(tc.tile_pool(name="rsp", bufs=2))
            stk = late.enter_context(tc.tile_pool(name="stk", bufs=1))
            outp = late.enter_context(tc.tile_pool(name="outp", bufs=1))

            # ---- qkv ----
            q_sb = [kqv.tile([128, TS], BF16, name=f"q{o}") for o in range(2)]
            k_sb = [kqv.tile([128, T], BF16, name=f"k{o}") for o in range(2)]
            vTa = kqv.tile([128, H, NCHUNKS, 65], BF16, name="vTa")
            nc.sync.dma_start(
                out=vTa[:, :, :, 64:65],
                in_=onesb_d.rearrange("p (a b one) -> p a b one", a=H, one=1),
            )
            for o in range(2):
                q_ps = psA.tile([128, TS], F32, name="q_ps", tag="big")
                for cc in range(2):
                    _mm(nc, q_ps, wq[cc][:, o * 128:(o + 1) * 128],
                        xn[cc][:, 0:TS], start=(cc == 0), stop=(cc == 1),
                        r=False)
                nc.vector.tensor_copy(q_sb[o], q_ps)
            for o in range(2):
                for nk in range(8):
                    k_ps = psA.tile([128, 512], F32, name="k_ps", tag="big")
                    for cc in range(2):
                        _mm(nc, k_ps, wk[cc][:, o * 128:(o + 1) * 128],
                            xn[cc][:, nk * 512:(nk + 1) * 512],
                            start=(cc == 0), stop=(cc == 1), r=False)
                    nc.vector.tensor_copy(k_sb[o][:, nk * 512:(nk + 1) * 512], k_ps)
            for tci in range(NCHUNKS):
                vt_ps = psA.tile([128, C], F32, name="vt_ps", tag="big")
                for cc in range(2):
                    _mm(nc, vt_ps, xn[cc][:, tci * 128:(tci + 1) * 128],
                        wv[cc], start=(cc == 0), stop=(cc == 1), r=False)
                nc.vector.tensor_copy(
                    vTa[:, :, tci, 0:64],
                    vt_ps.rearrange("p (h c) -> p h c", h=H),
                )

            # ---- attention (head pairs share k/q tiles; S^T layout) ----
            stacks = {}
            for pair in ((0, 1), (2, 3)):
                pv_ps = {}
                for h in pair:
                    pv_ps[h] = psB.tile([65, TS], F32, name=f"pv{h}", tag="acc")
                for sc in range(NCHUNKS):
                    p_t = {}
                    for h in pair:
                        kt = k_sb[h // 2]
                        qt = q_sb[h // 2]
                        lo = (h % 2) * 64
                        qk_ps = psA.tile([128, TS], F32, name="qk_ps", tag="big")
                        _mm(nc, qk_ps,
                            kt[lo:lo + 64, sc * 128:(sc + 1) * 128],
                            qt[lo:lo + 64, :], r=False)
                        p_t[h] = ppool.tile([128, TS], BF16, name="p_t", tag="p")
                        nc.scalar.activation(p_t[h], qk_ps, AF.Exp, scale=SCALE2)
                    for h in pair:
                        _mm(nc, pv_ps[h], vTa[:, h, sc, :], p_t[h],
                            start=(sc == 0), stop=(sc == NCHUNKS - 1), r=False)
                # normalize: stack_h = out2 / rowsum
                for h in pair:
                    rs_sb = rsp.tile([65, TS], F32R, name="rs_sb", tag="rs")
                    nc.scalar.copy(rs_sb[64:65, :], pv_ps[h][64:65, :])
                    bc_ps = psA.tile([64, TS], F32, name="bc_ps", tag="big")
                    _mm(nc, bc_ps, ones[64:65, 0:64], rs_sb[64:65, :])
                    recip = rsp.tile([64, TS], F32, name="recip", tag="recip")
                    nc.vector.reciprocal(recip, bc_ps)
                    stack = stk.tile([64, TS], BF16, name=f"stack{h}",
                                     tag=f"stack{h}")
                    nc.vector.tensor_mul(stack, pv_ps[h][0:64, :], recip)
                    stacks[h] = stack

            # ---- proj + bias + residual ----
            out_sb = outp.tile([128, 8, C], F32, name="out_sb")
            for tci in range(8):
                pr_ps = psB.tile([128, C], F32, name="pr_ps", tag="acc")
                for h in range(H):
                    _mm(nc, pr_ps, stacks[h][:, tci * 128:(tci + 1) * 128],
                        pT[h], start=(h == 0), stop=False, r=False)
                _mm(nc, pr_ps, ones[0:1, 0:128], projb,
                    start=False, stop=True)
                nc.vector.tensor_add(out_sb[:, tci, :], pr_ps, xT_sb[:, tci, :])
                nc.sync.dma_start(
                    out=yT_d[tci * 128:(tci + 1) * 128, :], in_=out_sb[:, tci, :]
                )

    # Legalize for this walrus: at most 1 sync wait per instruction.
    import bass_rust as _bass_rust
    _bass_rust.move_matmul_waits_to_ldweights(nc.m)
    _bass_rust.generate_event_semaphores(nc)
    return nc


def host_prep(inputs):
    """Per-core input dicts (pure slicing / transpose / permutation)."""
    x = np.ascontiguousarray(np.asarray(inputs["x"], np.float32).reshape(2, C, T))
    qkv_w = np.asarray(inputs["qkv_w"], np.float32)
    proj_w = np.asarray(inputs["proj_w"], np.float32)
    norm_w = np.ascontiguousarray(np.asarray(inputs["norm_w"], np.float32))
    norm_b = np.ascontiguousarray(np.asarray(inputs["norm_b"], np.float32))
    proj_b = np.ascontiguousarray(np.asarray(inputs["proj_b"], np.float32))

    q_idx = np.concatenate([np.arange(h * 192, h * 192 + 64) for h in range(H)])
    wqT = np.ascontiguousarray(qkv_w[q_idx].T)
    wkT = np.ascontiguousarray(qkv_w[q_idx + 64].T)
    wvT = np.ascontiguousarray(qkv_w[q_idx + 128].T)
    pT = np.ascontiguousarray(proj_w.T.reshape(4, 64, C))

    sel = np.zeros((128, 16), np.float32)
    sel[np.arange(128), np.arange(128) // 8] = 1.0 / 8.0
    expand = np.zeros((16, 128), np.float32)
    expand[np.arange(128) // 8, np.arange(128)] = 1.0

    bf = __import__("ml_dtypes").bfloat16
    shared = {
        "wqT": wqT.astype(bf), "wkT": wkT.astype(bf), "wvT": wvT.astype(bf),
        "pT": pT.astype(bf),
        "normw": np.ascontiguousarray(norm_w.reshape(2, 128, 1)),
        "normb": np.ascontiguousarray(norm_b.reshape(2, 128, 1)),
        "projb": np.ascontiguousarray(proj_b.reshape(1, C)),
        "sel": sel, "expand": expand,
        "ones": np.ones((128, 128), np.float32),
        "onesb": np.ones((128, 128), np.float32).astype(
            __import__("ml_dtypes").bfloat16),
    }
    in_maps = []
    for core in range(8):
        b, i = core // 4, core % 4
        t0 = i * TS
        m = dict(shared)
        m["x"] = np.ascontiguousarray(np.roll(x[b], -t0, axis=1))
        m["xT"] = np.ascontiguousarray(x[b, :, t0:t0 + TS].T)
        in_maps.append(m)
    return in_maps


def gather(core_outs):
    y = np.empty((2, C, T), np.float32)
    for core in range(8):
        b, i = core // 4, core % 4
        y[b, :, i * TS:(i + 1) * TS] = core_outs[core].T
    return y.reshape(2, C, 16, 16, 16)


_NC = None


def _get_nc():
    global _NC
    if _NC is None:
        _NC = build_nc()
    return _NC


def run(inputs, trace=False, trace_cores=None):
    nc = _get_nc()
    in_maps = host_prep(inputs)
    res = run_bass_kernel_spmd(
        nc, in_maps, list(range(8)), trace=trace, trace_cores=trace_cores
    )
    out = gather([res.results[c]["yT"] for c in range(8)])
    return out, res


def kernel(**inputs) -> np.ndarray:
    out, _ = run(inputs)
    return out



# revision 6
# speedup vs baseline: 1.2886x; 1.2886x over previous
"""Trainium2 Bass kernel for nn_AttentionBlock_15693810500077.

GroupNorm(32 groups) -> 1x1 qkv conv -> 4-head attention (T=4096) ->
1x1 proj -> residual, for x [2, 256, 16, 16, 16] fp32.

Sharding: 8 cores = (batch b in {0,1}) x (t-slice i in {0..3}, TS=1024).
Each core computes the full attention rows for its t-slice of its batch,
for all 4 heads, plus the projection and residual -> y^T slab [1024, 256].
The host rotates each core's x copy (np.roll over T) so the core's t-slice
always sits at columns 0:1024 -> one static SPMD program for all cores
(softmax over keys is permutation invariant).

v2 optimizations vs the 371us baseline:
- fp8e4 DoubleRow matmuls for qkv and P@V (2 contraction tiles of K=128
  per instruction); p and v quantized to fp8 (exp biased by -2.5 so
  p <= ~96 < 240 fp8e4 max; the bias cancels in the softmax normalize).
- exp split across the Activation engine (true Exp -> fp8) and the DVE
  (Schraudolph: p_bits = round(s*A + B) as uint8, bitcast fp8), keeping
  the scalar engine off the critical path.
- softmax reciprocal via Act exp(-ln(rowsum)) instead of the slow DVE
  InstReciprocal; rowsum comes free from a ones-column in the PV matmul.
- x shipped as bf16 (GN feeds fp8 anyway), xn computed on Act+GpSimd,
  proj bias pre-folded into the host-side xT residual slab.
- QK stays bf16 (K=64 contraction gets no DoubleRow benefit).
"""
import math
import os

import numpy as np

os.environ.setdefault("JAX_COMPILATION_CACHE_DIR", "/tmp/jaxcache")

import concourse.bass as bass
import concourse.tile as tile
from concourse import mybir
from concourse.bass_utils import run_bass_kernel_spmd

F32 = mybir.dt.float32
F32R = mybir.dt.float32r
BF16 = mybir.dt.bfloat16
F8 = mybir.dt.float8e4
U8 = mybir.dt.uint8
AF = mybir.ActivationFunctionType
ALU = mybir.AluOpType
DRM = mybir.MatmulPerfMode.DoubleRow

H = 4
C = 256
T = 4096
TS = 1024
EPS = 1e-5
SCALE2 = 0.125            # (1/sqrt(sqrt(64)))^2, applied inside exp
EBIAS = -2.5              # exp bias; cancels in normalize, keeps p in fp8 range
SCH_A = SCALE2 * 8.0 / math.log(2.0)            # Schraudolph slope
SCH_B = (7 * 8 - 0.3) + EBIAS * (8.0 / math.log(2.0))  # Schraudolph offset
NSP = 16                  # key-chunk pairs (32 chunks of 128)

# exp engine split: True -> DVE Schraudolph, False -> Act true exp.
# Index is (pair_idx*NSP + sp)*2 + h_in_pair over 64 groups of 4 chunks.
DVE_FRAC = 0.40
_ids = np.arange(64)
DVE_GROUP = ((_ids * DVE_FRAC) % 1.0) > (1.0 - DVE_FRAC)
DVE_GROUP = [bool(((i + 1) * DVE_FRAC) % 1.0 < DVE_FRAC) for i in range(64)]


def build_nc():
    nc = bass.Bass()

    xb_d = nc.dram_tensor("xb", [128, 2, T], BF16, kind="ExternalInput")
    xT_d = nc.dram_tensor("xTb", [TS, C], F32, kind="ExternalInput")
    wq_d = nc.dram_tensor("wq2", [128, 2, C], U8, kind="ExternalInput")
    wk_d = nc.dram_tensor("wk2", [128, 2, C], U8, kind="ExternalInput")
    wv_d = nc.dram_tensor("wv2", [128, 2, C], U8, kind="ExternalInput")
    pT_d = nc.dram_tensor("pT4", [64, H, C], BF16, kind="ExternalInput")
    normw_d = nc.dram_tensor("normw", [2, 128, 1], F32, kind="ExternalInput")
    normb_d = nc.dram_tensor("normb", [2, 128, 1], F32, kind="ExternalInput")
    sel_d = nc.dram_tensor("sel", [128, 16], F32, kind="ExternalInput")
    exp_d = nc.dram_tensor("expand", [16, 128], F32, kind="ExternalInput")
    ones_d = nc.dram_tensor("ones", [128, 128], F32R, kind="ExternalInput")
    yT_d = nc.dram_tensor("yT", [TS, C], F32, kind="ExternalOutput")

    import contextlib

    with tile.TileContext(nc) as tc:
        with (
            tc.tile_pool(name="consts", bufs=1) as consts,
            tc.tile_pool(name="gnp", bufs=2) as gnp,
            tc.tile_pool(name="kqv", bufs=1) as kqv,
            tc.tile_pool(name="psA", bufs=4, space="PSUM") as psA,
            tc.tile_pool(name="psB", bufs=2, space="PSUM") as psB,
            contextlib.ExitStack() as late,
        ):
            # ---- constant / weight loads ----
            wq2 = consts.tile([128, 2, C], U8, name="wq2")
            wk2 = consts.tile([128, 2, C], U8, name="wk2")
            wv2 = consts.tile([128, 2, C], U8, name="wv2")
            nc.sync.dma_start(out=wq2, in_=wq_d[:])
            nc.sync.dma_start(out=wk2, in_=wk_d[:])
            nc.sync.dma_start(out=wv2, in_=wv_d[:])
            pT4 = consts.tile([64, H, C], BF16, name="pT4")
            nc.sync.dma_start(out=pT4, in_=pT_d[:])
            normw = [consts.tile([128, 1], F32, name=f"nw{i}") for i in range(2)]
            normb = [consts.tile([128, 1], F32, name=f"nb{i}") for i in range(2)]
            for i in range(2):
                nc.sync.dma_start(out=normw[i], in_=normw_d[i])
                nc.sync.dma_start(out=normb[i], in_=normb_d[i])
            sel = consts.tile([128, 16], F32, name="sel")
            nc.sync.dma_start(out=sel, in_=sel_d[:])
            expand = consts.tile([16, 128], F32, name="expand")
            nc.sync.dma_start(out=expand, in_=exp_d[:])
            ones = consts.tile([128, 128], F32R, name="ones")
            nc.sync.dma_start(out=ones, in_=ones_d[:])
            xT_sb = consts.tile([128, 8, C], F32, name="xT_sb")
            nc.sync.dma_start(
                out=xT_sb, in_=xT_d.rearrange("(a p) o -> p a o", p=128)
            )
            ebias = consts.tile([128, 1], F32, name="ebias")
            nc.vector.memset(ebias, EBIAS)

            # ---- load x (bf16), GroupNorm -> xn fp8 ----
            xn2 = kqv.tile([128, 2, T], U8, name="xn2")
            with tc.tile_pool(name="xp", bufs=1) as xp:
                xb = xp.tile([128, 2, T], BF16, name="xb")
                for i in range(2):
                    for jc in range(4):
                        nc.sync.dma_start(
                            out=xb[:, i, jc * 1024:(jc + 1) * 1024],
                            in_=xb_d[:, i, jc * 1024:(jc + 1) * 1024],
                        )
                for i in range(2):
                    xv = xb[:, i, :].rearrange("p (a f) -> p a f", f=512)
                    stats = gnp.tile([128, 8, 6], F32, name="stats", tag="stats")
                    for j in range(8):
                        nc.vector.bn_stats(out=stats[:, j, :], in_=xv[:, j, :])
                    mv = gnp.tile([128, 2], F32, name="mv", tag="mv")
                    nc.vector.bn_aggr(out=mv, in_=stats)
                    msq = gnp.tile([128, 1], F32, name="msq", tag="msq")
                    nc.vector.tensor_mul(msq, mv[:, 0:1], mv[:, 0:1])
                    exsq = gnp.tile([128, 1], F32, name="exsq", tag="exsq")
                    nc.vector.tensor_add(exsq, msq, mv[:, 1:2])
                    gm_ps = psB.tile([16, 1], F32, name="gm_ps", tag="acc")
                    nc.tensor.matmul(gm_ps, sel, mv[:, 0:1], start=True, stop=True)
                    gx_ps = psB.tile([16, 1], F32, name="gx_ps", tag="acc")
                    nc.tensor.matmul(gx_ps, sel, exsq, start=True, stop=True)
                    gm_sb = gnp.tile([16, 1], F32, name="gm_sb", tag="gm_sb")
                    nc.vector.tensor_copy(gm_sb, gm_ps)
                    gmsq = gnp.tile([16, 1], F32, name="gmsq", tag="gmsq")
                    nc.vector.tensor_mul(gmsq, gm_sb, gm_sb)
                    gvar = gnp.tile([16, 1], F32, name="gvar", tag="gvar")
                    nc.vector.scalar_tensor_tensor(
                        gvar, gx_ps, EPS, gmsq, op0=ALU.add, op1=ALU.subtract
                    )
                    lnv = gnp.tile([16, 1], F32, name="lnv", tag="lnv")
                    nc.scalar.activation(lnv, gvar, AF.Ln)
                    rstd = gnp.tile([16, 1], F32, name="rstd", tag="rstd")
                    nc.scalar.activation(rstd, lnv, AF.Exp, scale=-0.5)
                    me_ps = psB.tile([128, 1], F32, name="me_ps", tag="acc")
                    nc.tensor.matmul(me_ps, expand, gm_sb, start=True, stop=True)
                    re_ps = psB.tile([128, 1], F32, name="re_ps", tag="acc")
                    nc.tensor.matmul(re_ps, expand, rstd, start=True, stop=True)
                    a_sb = gnp.tile([128, 1], F32, name="a_sb", tag=f"a_sb{i}")
                    nc.vector.tensor_mul(a_sb, re_ps, normw[i])
                    t2 = gnp.tile([128, 1], F32, name="t2", tag="t2")
                    nc.vector.tensor_mul(t2, me_ps, a_sb)
                    b_sb = gnp.tile([128, 1], F32, name="b_sb", tag=f"b_sb{i}")
                    nc.vector.tensor_sub(b_sb, normb[i], t2)
                    if i == 0:
                        # xn = a*x + b on the Act engine -> fp8
                        for jc in range(2):
                            sl = slice(jc * 2048, (jc + 1) * 2048)
                            nc.scalar.activation(
                                xn2[:, i, sl].bitcast(F8), xb[:, i, sl],
                                AF.Identity, bias=b_sb, scale=a_sb,
                            )
                    else:
                        # second block on GpSimd (runs concurrently)
                        nc.gpsimd.tensor_scalar(
                            out=xn2[:, i, :].bitcast(F8), in0=xb[:, i, :],
                            scalar1=a_sb, scalar2=b_sb,
                            op0=ALU.mult, op1=ALU.add,
                        )

            # ---- late pools ----
            ppool = late.enter_context(tc.tile_pool(name="ppool", bufs=4))
            rsp = late.enter_context(tc.tile_pool(name="rsp", bufs=2))
            stk = late.enter_context(tc.tile_pool(name="stk", bufs=1))
            outp = late.enter_context(tc.tile_pool(name="outp", bufs=1))

            # ---- qkv via fp8 DoubleRow (contraction 256 = 2 tiles of 128) ----
            q2 = kqv.tile([128, 2, TS], BF16, name="q2")
            k2 = kqv.tile([128, 2, T], BF16, name="k2")
            vT2 = kqv.tile([128, H, NSP, 2, 80], U8, name="vT2")
            nc.vector.memset(vT2[:, :, :, :, 64:65], 0x38)  # fp8e4 1.0 bits
            xn8 = xn2.bitcast(F8)
            for o in range(2):
                for nn in range(2):
                    sl = slice(nn * 512, (nn + 1) * 512)
                    q_ps = psA.tile([128, 512], F32, name="q_ps", tag="big")
                    nc.tensor.matmul(
                        q_ps, wq2.bitcast(F8)[:, :, o * 128:(o + 1) * 128],
                        xn8[:, :, sl], start=True, stop=True, perf_mode=DRM,
                    )
                    nc.vector.tensor_copy(q2[:, o, sl], q_ps)
            for o in range(2):
                for nk in range(8):
                    sl = slice(nk * 512, (nk + 1) * 512)
                    k_ps = psA.tile([128, 512], F32, name="k_ps", tag="big")
                    nc.tensor.matmul(
                        k_ps, wk2.bitcast(F8)[:, :, o * 128:(o + 1) * 128],
                        xn8[:, :, sl], start=True, stop=True, perf_mode=DRM,
                    )
                    nc.vector.tensor_copy(k2[:, o, sl], k_ps)
            for tci in range(2 * NSP):
                vt_ps = psB.tile([128, C], F32, name="vt_ps", tag="acc")
                nc.tensor.matmul(
                    vt_ps, xn8[:, :, tci * 128:(tci + 1) * 128],
                    wv2.bitcast(F8), start=True, stop=True, perf_mode=DRM,
                )
                nc.vector.tensor_copy(
                    vT2[:, :, tci // 2, tci % 2, 0:64].bitcast(F8),
                    vt_ps.rearrange("p (h c) -> p h c", h=H),
                )

            # ---- attention ----
            stacks = {}
            gi = 0
            for pair in ((0, 1), (2, 3)):
                pv_ps = {}
                for h in pair:
                    pv_ps[h] = psB.tile([65, TS], F32, name=f"pv{h}", tag="acc")
                for sp in range(NSP):
                    p2 = {}
                    for h in pair:
                        o, lo = h // 2, (h % 2) * 64
                        use_dve = DVE_GROUP[gi % 64]
                        gi += 1
                        p2[h] = ppool.tile([128, 2, TS], U8, name="p2", tag="p")
                        for half in range(2):
                            sc = sp * 2 + half
                            kt = k2[lo:lo + 64, o, sc * 128:(sc + 1) * 128]
                            for qh in range(2):
                                qs = slice(qh * 512, (qh + 1) * 512)
                                qk_ps = psA.tile([128, 512], F32, name="qk_ps",
                                                 tag="big")
                                nc.tensor.matmul(
                                    qk_ps, kt, q2[lo:lo + 64, o, qs],
                                    start=True, stop=True,
                                )
                                if use_dve:
                                    nc.vector.tensor_scalar(
                                        out=p2[h][:, half, qs], in0=qk_ps,
                                        scalar1=SCH_A, scalar2=SCH_B,
                                        op0=ALU.mult, op1=ALU.add,
                                    )
                                else:
                                    nc.scalar.activation(
                                        p2[h][:, half, qs].bitcast(F8), qk_ps,
                                        AF.Exp, scale=SCALE2, bias=ebias,
                                    )
                    for h in pair:
                        for qh in range(2):
                            qs = slice(qh * 512, (qh + 1) * 512)
                            nc.tensor.matmul(
                                pv_ps[h][:, qs],
                                vT2[:, h, sp, :, 0:65].bitcast(F8),
                                p2[h].bitcast(F8)[:, :, qs],
                                start=(sp == 0), stop=(sp == NSP - 1),
                                perf_mode=DRM,
                            )
                # normalize: stack_h = pv / rowsum via exp(-ln) on Act
                for h in pair:
                    lnr = rsp.tile([1, TS], F32, name="lnr", tag="lnr")
                    nc.scalar.activation(lnr, pv_ps[h][64:65, :], AF.Ln)
                    recip = rsp.tile([1, TS], F32R, name="recip", tag="recip")
                    nc.scalar.activation(recip, lnr, AF.Exp, scale=-1.0)
                    stack = stk.tile([64, TS], BF16, name=f"stack{h}",
                                     tag=f"stack{h}")
                    for qh in range(2):
                        qs = slice(qh * 512, (qh + 1) * 512)
                        bc_ps = psA.tile([64, 512], F32, name="bc_ps",
                                         tag="big")
                        nc.tensor.matmul(bc_ps, ones[0:1, 0:64],
                                         recip[:, qs], start=True, stop=True)
                        bcs = rsp.tile([64, 512], F32, name="bcs", tag="bcs")
                        nc.vector.tensor_copy(bcs, bc_ps)
                        nc.vector.tensor_mul(stack[:, qs], pv_ps[h][0:64, qs],
                                             bcs)
                    stacks[h] = stack

            # ---- proj + residual (bias pre-folded into xT) ----
            out_sb = outp.tile([128, 8, C], F32, name="out_sb")
            for tci in range(8):
                pr_ps = psB.tile([128, C], F32, name="pr_ps", tag="acc")
                for h in range(H):
                    nc.tensor.matmul(
                        pr_ps, stacks[h][:, tci * 128:(tci + 1) * 128],
                        pT4[:, h, :], start=(h == 0), stop=(h == H - 1),
                    )
                nc.vector.tensor_add(out_sb[:, tci, :], pr_ps, xT_sb[:, tci, :])
                nc.sync.dma_start(
                    out=yT_d[tci * 128:(tci + 1) * 128, :], in_=out_sb[:, tci, :]
                )

    import bass_rust as _bass_rust
    _bass_rust.move_matmul_waits_to_ldweights(nc.m)
    _bass_rust.generate_event_semaphores(nc)
    return nc


def host_prep(inputs):
    """Per-core input dicts (slicing / transpose / dtype packing only)."""
    import ml_dtypes
    bf = ml_dtypes.bfloat16
    f8 = ml_dtypes.float8_e4m3

    x = np.ascontiguousarray(np.asarray(inputs["x"], np.float32).reshape(2, C, T))
    qkv_w = np.asarray(inputs["qkv_w"], np.float32)
    proj_w = np.asarray(inputs["proj_w"], np.float32)
    norm_w = np.ascontiguousarray(np.asarray(inputs["norm_w"], np.float32))
    norm_b = np.ascontiguousarray(np.asarray(inputs["norm_b"], np.float32))
    proj_b = np.ascontiguousarray(np.asarray(inputs["proj_b"], np.float32))

    # qkv channel order: head-major (h0 d0..63, h1 d0..63, ...)
    q_idx = np.concatenate([np.arange(h * 192, h * 192 + 64) for h in range(H)])
    wqT = qkv_w[q_idx].T          # [C_in, C_out]
    wkT = qkv_w[q_idx + 64].T
    wvT = qkv_w[q_idx + 128].T
    # DoubleRow layout: [128, 2 (input-ch tile), C_out] fp8 bits
    def dr_pack(wT):
        return np.ascontiguousarray(
            wT.reshape(2, 128, C).transpose(1, 0, 2).astype(f8)).view(np.uint8)
    pT4 = np.ascontiguousarray(
        proj_w.T.reshape(H, 64, C).transpose(1, 0, 2)).astype(bf)

    sel = np.zeros((128, 16), np.float32)
    sel[np.arange(128), np.arange(128) // 8] = 1.0 / 8.0
    expand = np.zeros((16, 128), np.float32)
    expand[np.arange(128) // 8, np.arange(128)] = 1.0

    shared = {
        "wq2": dr_pack(wqT), "wk2": dr_pack(wkT), "wv2": dr_pack(wvT),
        "pT4": pT4,
        "normw": np.ascontiguousarray(norm_w.reshape(2, 128, 1)),
        "normb": np.ascontiguousarray(norm_b.reshape(2, 128, 1)),
        "sel": sel, "expand": expand,
        "ones": np.ones((128, 128), np.float32),
    }
    in_maps = []
    for core in range(8):
        b, i = core // 4, core % 4
        t0 = i * TS
        m = dict(shared)
        xr = np.roll(x[b], -t0, axis=1)
        m["xb"] = np.ascontiguousarray(xr.reshape(2, 128, T).transpose(1, 0, 2)
                                       ).astype(bf)
        m["xTb"] = np.ascontiguousarray(x[b, :, t0:t0 + TS].T
                                        + proj_b[None, :])
        in_maps.append(m)
    return in_maps


def gather(core_outs):
    y = np.empty((2, C, T), np.float32)
    for core in range(8):
        b, i = core // 4, core % 4
        y[b, :, i * TS:(i + 1) * TS] = core_outs[core].T
    return y.reshape(2, C, 16, 16, 16)


_NC = None


def _get_nc():
    global _NC
    if _NC is None:
        _NC = build_nc()
    return _NC


def run(inputs, trace=False, trace_cores=None):
    nc = _get_nc()
    in_maps = host_prep(inputs)
    res = run_bass_kernel_spmd(
        nc, in_maps, list(range(8)), trace=trace, trace_cores=trace_cores
    )
    out = gather([res.results[c]["yT"] for c in range(8)])
    return out, res


def kernel(**inputs) -> np.ndarray:
    out, _ = run(inputs)
    return out


# revision 9
# speedup vs baseline: 1.3172x; 1.0222x over previous
"""Trainium2 Bass kernel for nn_AttentionBlock_15693810500077.

GroupNorm(32 groups) -> 1x1 qkv conv -> 4-head attention (T=4096) ->
1x1 proj -> residual, for x [2, 256, 16, 16, 16] fp32.

Sharding: 8 cores = (batch b in {0,1}) x (t-slice i in {0..3}, TS=1024).
Each core computes the full attention rows for its t-slice of its batch,
for all 4 heads, plus the projection and residual -> y^T slab [1024, 256].
The host rotates each core's x copy (np.roll over T) so the core's t-slice
always sits at columns 0:1024 -> one static SPMD program for all cores
(softmax over keys is permutation invariant).

v3: keeps the PE gap-free so the HAM clock gate stays at 8/8 (2.4 GHz):
- one head at a time (pv accumulator = 2 PSUM banks) with software
  pipelining: PV of iteration sp-1 is emitted between the QK groups of
  iteration sp, so the in-order PE queue never stalls on exp.
- fp8e4 DoubleRow matmuls for qkv and P@V; exp is biased by -2.5 so
  p fits fp8 (bias cancels in the softmax normalize).
- exp on [128,1024] tiles, split between Act (true Exp -> fp8) and DVE
  (Schraudolph: round(s*A+B) -> uint8 = fp8 bits).
- softmax 1/rowsum via Act exp(-ln(rowsum)); rowsum comes free from a
  ones-column in the PV matmul. pv is copied PSUM->SBUF right after the
  accumulation stops so the single pv bank frees for the next head and
  the normalize overlaps the next head's attention.
- v^T production is interleaved into head-0's loop (chunk pair sp+1
  produced during iteration sp).
- x ships as bf16; xn computed on Act+GpSimd straight to fp8; proj bias
  pre-folded into the host-side xT residual slab; QK stays bf16.
"""
import math
import os

import numpy as np

os.environ.setdefault("JAX_COMPILATION_CACHE_DIR", "/tmp/jaxcache")

import concourse.bass as bass
import concourse.tile as tile
from concourse import mybir
from concourse.bass_utils import run_bass_kernel_spmd

F32 = mybir.dt.float32
F32R = mybir.dt.float32r
BF16 = mybir.dt.bfloat16
F8 = mybir.dt.float8e4
U8 = mybir.dt.uint8
AF = mybir.ActivationFunctionType
ALU = mybir.AluOpType
DRM = mybir.MatmulPerfMode.DoubleRow

H = 4
C = 256
T = 4096
TS = 1024
EPS = 1e-5
SCALE2 = 0.125            # (1/sqrt(sqrt(64)))^2, applied inside exp
EBIAS = -2.5              # keeps p <= ~96 < 240 (fp8e4 max); cancels in norm
SCH_A = SCALE2 * 8.0 / math.log(2.0)
SCH_B = (7 * 8 - 0.3) + EBIAS * (8.0 / math.log(2.0))
NSP = 16                  # chunk pairs (32 key chunks of 128)

# exp engine split per (head, chunk): True -> DVE Schraudolph, else Act Exp.
DVE_FRAC = 0.5


def _use_dve(idx):
    return (int((idx + 1) * DVE_FRAC) - int(idx * DVE_FRAC)) > 0


def build_nc():
    nc = bass.Bass()

    xb_d = nc.dram_tensor("xb", [128, 2, T], BF16, kind="ExternalInput")
    xT_d = nc.dram_tensor("xTb", [TS, C], F32, kind="ExternalInput")
    wq_d = nc.dram_tensor("wq2", [128, 2, C], U8, kind="ExternalInput")
    wk_d = nc.dram_tensor("wk2", [128, 2, C], U8, kind="ExternalInput")
    wv_d = nc.dram_tensor("wv2", [128, 2, C], U8, kind="ExternalInput")
    pT_d = nc.dram_tensor("pT4", [64, H, C], BF16, kind="ExternalInput")
    normw_d = nc.dram_tensor("normw", [2, 128, 1], F32, kind="ExternalInput")
    normb_d = nc.dram_tensor("normb", [2, 128, 1], F32, kind="ExternalInput")
    sel_d = nc.dram_tensor("sel", [128, 16], F32, kind="ExternalInput")
    exp_d = nc.dram_tensor("expand", [16, 128], F32, kind="ExternalInput")
    ones_d = nc.dram_tensor("ones", [128, 128], F32R, kind="ExternalInput")
    yT_d = nc.dram_tensor("yT", [TS, C], F32, kind="ExternalOutput")

    import contextlib

    with tile.TileContext(nc) as tc:
        with (
            tc.tile_pool(name="consts", bufs=1) as consts,
            tc.tile_pool(name="gnp", bufs=2) as gnp,
            tc.tile_pool(name="kqv", bufs=1) as kqv,
            tc.tile_pool(name="psA", bufs=2, space="PSUM") as psA,
            tc.tile_pool(name="psPV", bufs=1, space="PSUM") as psPV,
            tc.tile_pool(name="psB", bufs=2, space="PSUM") as psB,
            contextlib.ExitStack() as late,
        ):
            # ---- constant / weight loads ----
            wq2 = consts.tile([128, 2, C], U8, name="wq2")
            wk2 = consts.tile([128, 2, C], U8, name="wk2")
            wv2 = consts.tile([128, 2, C], U8, name="wv2")
            nc.sync.dma_start(out=wq2, in_=wq_d[:])
            nc.sync.dma_start(out=wk2, in_=wk_d[:])
            nc.sync.dma_start(out=wv2, in_=wv_d[:])
            pT4 = consts.tile([64, H, C], BF16, name="pT4")
            nc.sync.dma_start(out=pT4, in_=pT_d[:])
            normw = [consts.tile([128, 1], F32, name=f"nw{i}") for i in range(2)]
            normb = [consts.tile([128, 1], F32, name=f"nb{i}") for i in range(2)]
            for i in range(2):
                nc.sync.dma_start(out=normw[i], in_=normw_d[i])
                nc.sync.dma_start(out=normb[i], in_=normb_d[i])
            sel = consts.tile([128, 16], F32, name="sel")
            nc.sync.dma_start(out=sel, in_=sel_d[:])
            expand = consts.tile([16, 128], F32, name="expand")
            nc.sync.dma_start(out=expand, in_=exp_d[:])
            ones = consts.tile([128, 128], F32R, name="ones")
            nc.sync.dma_start(out=ones, in_=ones_d[:])
            xT_sb = consts.tile([128, 8, C], F32, name="xT_sb")
            nc.sync.dma_start(
                out=xT_sb, in_=xT_d.rearrange("(a p) o -> p a o", p=128)
            )
            ebias = consts.tile([128, 1], F32, name="ebias")
            nc.vector.memset(ebias, EBIAS)

            # ---- load x (bf16), GroupNorm -> xn fp8 ----
            xn2 = kqv.tile([128, 2, T], U8, name="xn2")
            with tc.tile_pool(name="xp", bufs=1) as xp:
                xb = xp.tile([128, 2, T], BF16, name="xb")
                for i in range(2):
                    for jc in range(4):
                        nc.sync.dma_start(
                            out=xb[:, i, jc * 1024:(jc + 1) * 1024],
                            in_=xb_d[:, i, jc * 1024:(jc + 1) * 1024],
                        )
                for i in range(2):
                    xv = xb[:, i, :].rearrange("p (a f) -> p a f", f=512)
                    stats = gnp.tile([128, 8, 6], F32, name="stats", tag="stats")
                    for j in range(8):
                        nc.vector.bn_stats(out=stats[:, j, :], in_=xv[:, j, :])
                    mv = gnp.tile([128, 2], F32, name="mv", tag="mv")
                    nc.vector.bn_aggr(out=mv, in_=stats)
                    msq = gnp.tile([128, 1], F32, name="msq", tag="msq")
                    nc.vector.tensor_mul(msq, mv[:, 0:1], mv[:, 0:1])
                    exsq = gnp.tile([128, 1], F32, name="exsq", tag="exsq")
                    nc.vector.tensor_add(exsq, msq, mv[:, 1:2])
                    gm_ps = psB.tile([16, 1], F32, name="gm_ps", tag="vt")
                    nc.tensor.matmul(gm_ps, sel, mv[:, 0:1], start=True, stop=True)
                    gx_ps = psB.tile([16, 1], F32, name="gx_ps", tag="vt")
                    nc.tensor.matmul(gx_ps, sel, exsq, start=True, stop=True)
                    gm_sb = gnp.tile([16, 1], F32, name="gm_sb", tag="gm_sb")
                    nc.vector.tensor_copy(gm_sb, gm_ps)
                    gmsq = gnp.tile([16, 1], F32, name="gmsq", tag="gmsq")
                    nc.vector.tensor_mul(gmsq, gm_sb, gm_sb)
                    gvar = gnp.tile([16, 1], F32, name="gvar", tag="gvar")
                    nc.vector.scalar_tensor_tensor(
                        gvar, gx_ps, EPS, gmsq, op0=ALU.add, op1=ALU.subtract
                    )
                    lnv = gnp.tile([16, 1], F32, name="lnv", tag="lnv")
                    nc.scalar.activation(lnv, gvar, AF.Ln)
                    rstd = gnp.tile([16, 1], F32, name="rstd", tag="rstd")
                    nc.scalar.activation(rstd, lnv, AF.Exp, scale=-0.5)
                    me_ps = psB.tile([128, 1], F32, name="me_ps", tag="vt")
                    nc.tensor.matmul(me_ps, expand, gm_sb, start=True, stop=True)
                    re_ps = psB.tile([128, 1], F32, name="re_ps", tag="vt")
                    nc.tensor.matmul(re_ps, expand, rstd, start=True, stop=True)
                    a_sb = gnp.tile([128, 1], F32, name="a_sb", tag=f"a_sb{i}")
                    nc.vector.tensor_mul(a_sb, re_ps, normw[i])
                    t2 = gnp.tile([128, 1], F32, name="t2", tag="t2")
                    nc.vector.tensor_mul(t2, me_ps, a_sb)
                    b_sb = gnp.tile([128, 1], F32, name="b_sb", tag=f"b_sb{i}")
                    nc.vector.tensor_sub(b_sb, normb[i], t2)
                    if i == 0:
                        for jc in range(2):
                            sl = slice(jc * 2048, (jc + 1) * 2048)
                            nc.scalar.activation(
                                xn2[:, i, sl].bitcast(F8), xb[:, i, sl],
                                AF.Identity, bias=b_sb, scale=a_sb,
                            )
                    else:
                        nc.gpsimd.tensor_scalar(
                            out=xn2[:, i, :].bitcast(F8), in0=xb[:, i, :],
                            scalar1=a_sb, scalar2=b_sb,
                            op0=ALU.mult, op1=ALU.add,
                        )

            # ---- late pools ----
            ppool = late.enter_context(tc.tile_pool(name="ppool", bufs=3))
            rsp = late.enter_context(tc.tile_pool(name="rsp", bufs=2))
            stk = late.enter_context(tc.tile_pool(name="stk", bufs=1))
            outp = late.enter_context(tc.tile_pool(name="outp", bufs=1))

            # ---- q, k via fp8 DoubleRow; v produced lazily in head-0 pass ----
            q2 = kqv.tile([128, 2, TS], BF16, name="q2")
            k2 = kqv.tile([128, 2, T], BF16, name="k2")
            vT2 = kqv.tile([128, H, NSP, 2, 80], U8, name="vT2")
            nc.vector.memset(vT2[:, :, :, :, 64:65], 0x38)  # fp8e4 1.0 bits
            xn8 = xn2.bitcast(F8)
            for o in range(2):
                q_ps = psA.tile([128, TS], F32, name="q_ps", tag="big")
                for nn in range(2):
                    sl = slice(nn * 512, (nn + 1) * 512)
                    nc.tensor.matmul(
                        q_ps[:, sl],
                        wq2.bitcast(F8)[:, :, o * 128:(o + 1) * 128],
                        xn8[:, :, sl], start=True, stop=True, perf_mode=DRM,
                    )
                nc.vector.tensor_copy(q2[:, o, :], q_ps)
            for o in range(2):
                for nkp in range(4):
                    k_ps = psA.tile([128, TS], F32, name="k_ps", tag="big")
                    for nn in range(2):
                        nk = nkp * 2 + nn
                        sl = slice(nk * 512, (nk + 1) * 512)
                        nc.tensor.matmul(
                            k_ps[:, nn * 512:(nn + 1) * 512],
                            wk2.bitcast(F8)[:, :, o * 128:(o + 1) * 128],
                            xn8[:, :, sl], start=True, stop=True, perf_mode=DRM,
                        )
                    sl2 = slice(nkp * 1024, (nkp + 1) * 1024)
                    if nkp % 2 == 0:
                        nc.vector.tensor_copy(k2[:, o, sl2], k_ps)
                    else:
                        nc.scalar.copy(k2[:, o, sl2], k_ps)

            def make_v(pair_i):
                """Produce v^T chunk pair pair_i (two 128-key chunks)."""
                for half in range(2):
                    tci = pair_i * 2 + half
                    vt_ps = psB.tile([128, C], F32, name="vt_ps", tag="vt")
                    nc.tensor.matmul(
                        vt_ps, xn8[:, :, tci * 128:(tci + 1) * 128],
                        wv2.bitcast(F8), start=True, stop=True, perf_mode=DRM,
                    )
                    dst = vT2[:, :, pair_i, half, 0:64].bitcast(F8)
                    src = vt_ps.rearrange("p (h c) -> p h c", h=H)
                    if half == 0:
                        nc.vector.tensor_copy(dst, src)
                    else:
                        nc.scalar.copy(dst, src)

            make_v(0)
            make_v(1)

            # ---- attention: one head per pass, PV pipelined one sp behind ----
            stacks = {}
            ei = 0
            for h in range(H):
                o, lo = h // 2, (h % 2) * 64
                pv_ps = psPV.tile([65, TS], F32, name=f"pv{h}", tag="pv")
                p_hist = []
                for sp in range(NSP):
                    p2 = ppool.tile([128, 2, TS], U8, name="p2", tag="p")
                    for half in range(2):
                        sc = sp * 2 + half
                        kt = k2[lo:lo + 64, o, sc * 128:(sc + 1) * 128]
                        qk_ps = psA.tile([128, TS], F32, name="qk_ps", tag="big")
                        for qh in range(2):
                            qs = slice(qh * 512, (qh + 1) * 512)
                            nc.tensor.matmul(
                                qk_ps[:, qs], kt, q2[lo:lo + 64, o, qs],
                                start=True, stop=True,
                            )
                        if _use_dve(ei):
                            nc.vector.tensor_scalar(
                                out=p2[:, half, :], in0=qk_ps,
                                scalar1=SCH_A, scalar2=SCH_B,
                                op0=ALU.mult, op1=ALU.add,
                            )
                        else:
                            nc.scalar.activation(
                                p2[:, half, :].bitcast(F8), qk_ps,
                                AF.Exp, scale=SCALE2, bias=ebias,
                            )
                        ei += 1
                    p_hist.append(p2)
                    # lazily produce v chunk pair sp+1 during head 0
                    if h == 0 and sp + 2 < NSP:
                        make_v(sp + 2)
                    # PV for iteration sp-1 (its exps are long done)
                    if sp > 0:
                        pprev = p_hist[sp - 1]
                        for qh in range(2):
                            qs = slice(qh * 512, (qh + 1) * 512)
                            nc.tensor.matmul(
                                pv_ps[:, qs],
                                vT2[:, h, sp - 1, :, 0:65].bitcast(F8),
                                pprev.bitcast(F8)[:, :, qs],
                                start=(sp == 1), stop=False,
                                perf_mode=DRM,
                            )
                for qh in range(2):
                    qs = slice(qh * 512, (qh + 1) * 512)
                    nc.tensor.matmul(
                        pv_ps[:, qs],
                        vT2[:, h, NSP - 1, :, 0:65].bitcast(F8),
                        p_hist[NSP - 1].bitcast(F8)[:, :, qs],
                        start=False, stop=True, perf_mode=DRM,
                    )
                # free the pv bank fast, normalize from SBUF (overlaps next head)
                pvs = rsp.tile([65, TS], F32, name="pvs", tag="pvs")
                nc.vector.tensor_copy(pvs, pv_ps)
                lnr = rsp.tile([1, TS], F32, name="lnr", tag="lnr")
                nc.scalar.activation(lnr, pvs[64:65, :], AF.Ln)
                recip = rsp.tile([1, TS], F32R, name="recip", tag="recip")
                nc.scalar.activation(recip, lnr, AF.Exp, scale=-1.0)
                stack = stk.tile([64, TS], BF16, name=f"stack{h}",
                                 tag=f"stack{h}")
                for qh in range(2):
                    qs = slice(qh * 512, (qh + 1) * 512)
                    bc_big = psA.tile([128, TS], F32, name="bc", tag="big")
                    nc.tensor.matmul(bc_big[0:64, 0:512], ones[0:1, 0:64],
                                     recip[:, qs], start=True, stop=True)
                    nc.vector.tensor_mul(stack[:, qs], pvs[0:64, qs],
                                         bc_big[0:64, 0:512])
                stacks[h] = stack

            # ---- proj + residual (bias pre-folded into xT) ----
            out_sb = outp.tile([128, 8, C], F32, name="out_sb")
            for tci in range(8):
                pr_ps = psB.tile([128, C], F32, name="pr_ps", tag="vt")
                for h in range(H):
                    nc.tensor.matmul(
                        pr_ps, stacks[h][:, tci * 128:(tci + 1) * 128],
                        pT4[:, h, :], start=(h == 0), stop=(h == H - 1),
                    )
                nc.vector.tensor_add(out_sb[:, tci, :], pr_ps, xT_sb[:, tci, :])
                nc.sync.dma_start(
                    out=yT_d[tci * 128:(tci + 1) * 128, :], in_=out_sb[:, tci, :]
                )

    import bass_rust as _bass_rust
    _bass_rust.move_matmul_waits_to_ldweights(nc.m)
    _bass_rust.generate_event_semaphores(nc)
    return nc


def host_prep(inputs):
    """Per-core input dicts (slicing / transpose / dtype packing only)."""
    import ml_dtypes
    bf = ml_dtypes.bfloat16
    f8 = ml_dtypes.float8_e4m3

    x = np.ascontiguousarray(np.asarray(inputs["x"], np.float32).reshape(2, C, T))
    qkv_w = np.asarray(inputs["qkv_w"], np.float32)
    proj_w = np.asarray(inputs["proj_w"], np.float32)
    norm_w = np.ascontiguousarray(np.asarray(inputs["norm_w"], np.float32))
    norm_b = np.ascontiguousarray(np.asarray(inputs["norm_b"], np.float32))
    proj_b = np.ascontiguousarray(np.asarray(inputs["proj_b"], np.float32))

    q_idx = np.concatenate([np.arange(h * 192, h * 192 + 64) for h in range(H)])
    wqT = qkv_w[q_idx].T
    wkT = qkv_w[q_idx + 64].T
    wvT = qkv_w[q_idx + 128].T

    def dr_pack(wT):
        return np.ascontiguousarray(
            wT.reshape(2, 128, C).transpose(1, 0, 2).astype(f8)).view(np.uint8)

    pT4 = np.ascontiguousarray(
        proj_w.T.reshape(H, 64, C).transpose(1, 0, 2)).astype(bf)

    sel = np.zeros((128, 16), np.float32)
    sel[np.arange(128), np.arange(128) // 8] = 1.0 / 8.0
    expand = np.zeros((16, 128), np.float32)
    expand[np.arange(128) // 8, np.arange(128)] = 1.0

    shared = {
        "wq2": dr_pack(wqT), "wk2": dr_pack(wkT), "wv2": dr_pack(wvT),
        "pT4": pT4,
        "normw": np.ascontiguousarray(norm_w.reshape(2, 128, 1)),
        "normb": np.ascontiguousarray(norm_b.reshape(2, 128, 1)),
        "sel": sel, "expand": expand,
        "ones": np.ones((128, 128), np.float32),
    }
    in_maps = []
    for core in range(8):
        b, i = core // 4, core % 4
        t0 = i * TS
        m = dict(shared)
        xr = np.roll(x[b], -t0, axis=1)
        m["xb"] = np.ascontiguousarray(xr.reshape(2, 128, T).transpose(1, 0, 2)
                                       ).astype(bf)
        m["xTb"] = np.ascontiguousarray(x[b, :, t0:t0 + TS].T
                                        + proj_b[None, :])
        in_maps.append(m)
    return in_maps


def gather(core_outs):
    y = np.empty((2, C, T), np.float32)
    for core in range(8):
        b, i = core // 4, core % 4
        y[b, :, i * TS:(i + 1) * TS] = core_outs[core].T
    return y.reshape(2, C, 16, 16, 16)


_NC = None


def _get_nc():
    global _NC
    if _NC is None:
        _NC = build_nc()
    return _NC


def run(inputs, trace=False, trace_cores=None):
    nc = _get_nc()
    in_maps = host_prep(inputs)
    res = run_bass_kernel_spmd(
        nc, in_maps, list(range(8)), trace=trace, trace_cores=trace_cores
    )
    out = gather([res.results[c]["yT"] for c in range(8)])
    return out, res


def kernel(**inputs) -> np.ndarray:
    out, _ = run(inputs)
    return out
